# revision 1
# baseline (speedup 1.0000x reference)
"""CNOT (12-wire, dim 2) applied to a batch of state columns: out = U @ x.

U is a 0/1 permutation matrix (the dense CNOT gate), so U @ x is a pure row
permutation of x.  We verify that property on the actual U input, derive the
permutation, and compress it into maximal contiguous row blocks.  For the
CNOT these are: rows [0,2048) identity, rows [2048,3072) <-> [3072,4096)
swapped.

Execution strategy (data-parallel over batch columns, 8 NeuronCores, no
comms):

1. Primary: donate the per-core x device buffer so XLA aliases it to the
   output y (same shape/dtype).  Identity rows then need no data movement at
   all; only the moving blocks are staged DRAM -> SBUF -> DRAM (8MB instead
   of 32MB of HBM traffic per core).
2. If the aliasing is not honored (detected by checking identity rows of the
   returned output against x), fall back to a full-copy DRAM->DRAM
   permutation kernel, which writes every output row.
"""

import contextlib

import numpy as np

N_CORES = 8
# ~0.5MB staging chunks (rows per chunk of a moving block; measured best)
CHUNK_ROWS = 128
# moving payload must fit in SBUF (<= ~20MB of 24MB)
SBUF_BUDGET_BYTES = 20 * 2**20

_RUNNERS = {}


def _perm_to_blocks(perm):
    """Compress out[i] = x[perm[i]] into (dst_start, src_start, count) runs."""
    n = len(perm)
    blocks = []
    i = 0
    while i < n:
        j = int(perm[i])
        k = i + 1
        while k < n and int(perm[k]) == j + (k - i):
            k += 1
        blocks.append((i, j, k - i))
        i = k
    return tuple(blocks)


def _involution_pairs(moving):
    """If the moving blocks form disjoint swap pairs (dst_i==src_j,
    src_i==dst_j, cnt_i==cnt_j), return the pair list, else None."""
    by_key = {(d, s, c): (d, s, c) for d, s, c in moving}
    pairs, used = [], set()
    for d, s, c in moving:
        if (d, s, c) in used:
            continue
        partner = by_key.get((s, d, c))
        if partner is None:
            return None
        used.add((d, s, c))
        used.add((s, d, c))
        pairs.append(((d, s, c), (s, d, c)))
    return pairs


def _build_swap_nc(blocks, n, shard):
    """In-place program: x and y are runtime-aliased to the same buffer.

    Moving blocks are staged DRAM -> SBUF -> DRAM; identity blocks are
    untouched (the aliased buffer already holds their data).  Loads go on the
    sync HWDGE ring, stores on the scalar HWDGE ring.

    Schedule: if the moving blocks form an involution (disjoint swap pairs,
    as for the CNOT), chunk each pair and gate each store on only its own
    pair-chunk's loads, so the store stream trails the load stream by one
    chunk (~10% faster than a full barrier).  Otherwise fall back to a full
    load/store barrier, correct for any row permutation.
    """
    import concourse.bass as bass
    from concourse import mybir

    nc = bass.Bass(trn_type="TRN2")
    x = nc.dram_tensor("x", [n, shard], mybir.dt.float32, kind="ExternalInput")
    y = nc.dram_tensor("y", [n, shard], mybir.dt.float32, kind="ExternalOutput")

    moving = [(d, s, c) for d, s, c in blocks if d != s]
    pairs = _involution_pairs(moving)

    if pairs is not None:
        # interleaved pairwise schedule:
        # loads:  A_p0, B_p0, A_p1, B_p1, ...  (chunks of each pair adjacent)
        # stores: pair-chunk k waits until load 2k+2 complete
        def chunk_sizes(cnt):
            # small edge chunks: the first store starts after only 2 small
            # loads (ramp-in) and the final lone store drains quickly
            # (ramp-out); steady state uses full CHUNK_ROWS chunks.
            head = [s for s in (32, 64) if s < CHUNK_ROWS]
            sizes, rem = [], cnt
            for s in head:
                if rem >= s + CHUNK_ROWS + head[0]:
                    sizes.append(s)
                    rem -= s
            tail = head[0] if rem > CHUNK_ROWS and rem % CHUNK_ROWS == 0 else None
            if tail:
                rem -= tail
            while rem > 0:
                rc = min(CHUNK_ROWS, rem)
                sizes.append(rc)
                rem -= rc
            if tail:
                sizes.append(tail)
            return sizes

        load_list = []  # (src_row, rc)  in issue order
        store_list = []  # (dst_row, rc, tile_idx, loads_needed)
        for (d1, s1, c1), (d2, s2, c2) in pairs:
            r = 0
            for rc in chunk_sizes(c1):
                li = len(load_list)
                load_list.append((s1 + r, rc))  # tile li:   x[s1+r] -> y[d1+r]
                load_list.append((s2 + r, rc))  # tile li+1: x[s2+r] -> y[d2+r]
                store_list.append((d1 + r, rc, li, li + 2))
                store_list.append((d2 + r, rc, li + 1, li + 2))
                r += rc

        with contextlib.ExitStack() as ctx:
            tiles = [
                ctx.enter_context(
                    nc.sbuf_tensor(
                        f"t{i}", [128, rc * shard // 128], mybir.dt.float32
                    )
                )
                for i, (_, rc) in enumerate(load_list)
            ]
            sem_l = ctx.enter_context(nc.semaphore("sem_l"))
            sem_s = ctx.enter_context(nc.semaphore("sem_s"))
            block = ctx.enter_context(nc.Block())

            @block.sync
            def _(sync):
                for t, (src_row, rc) in zip(tiles, load_list):
                    sync.dma_start(
                        t[:, :], x[src_row : src_row + rc, :]
                    ).then_inc(sem_l, 16)

            @block.scalar
            def _(scalar):
                for dst_row, rc, ti, need_loads in store_list:
                    scalar.wait_ge(sem_l, 16 * need_loads)
                    scalar.dma_start(
                        y[dst_row : dst_row + rc, :], tiles[ti][:, :]
                    ).then_inc(sem_s, 16)
                scalar.wait_ge(sem_s, 16 * len(store_list))

        return nc

    # general fallback: full barrier between all loads and all stores
    chunks = []  # (dst_row, src_row, rows)
    for dst, src, cnt in moving:
        r = 0
        while r < cnt:
            rc = min(CHUNK_ROWS, cnt - r)
            chunks.append((dst + r, src + r, rc))
            r += rc

    with contextlib.ExitStack() as ctx:
        tiles = [
            ctx.enter_context(
                nc.sbuf_tensor(f"t{i}", [128, rc * shard // 128], mybir.dt.float32)
            )
            for i, (_, _, rc) in enumerate(chunks)
        ]
        sem_l = ctx.enter_context(nc.semaphore("sem_l"))
        sem_s = ctx.enter_context(nc.semaphore("sem_s"))
        block = ctx.enter_context(nc.Block())

        @block.sync
        def _(sync):
            for t, (_, src_row, rc) in zip(tiles, chunks):
                sync.dma_start(t[:, :], x[src_row : src_row + rc, :]).then_inc(
                    sem_l, 16
                )

        @block.scalar
        def _(scalar):
            # all loads done before any store: safe for arbitrary permutations
            scalar.wait_ge(sem_l, 16 * len(chunks))
            for t, (dst_row, _, rc) in zip(tiles, chunks):
                scalar.dma_start(y[dst_row : dst_row + rc, :], t[:, :]).then_inc(
                    sem_s, 16
                )
            scalar.wait_ge(sem_s, 16 * len(chunks))

    return nc


def _build_copy_nc(blocks, n, shard):
    """Full-copy fallback: y[dst:dst+cnt] = x[src:src+cnt] per block, pure
    DRAM->DRAM DMA.  Writes every output row; x and y are separate buffers."""
    import concourse.bass as bass
    from concourse import mybir

    nc = bass.Bass(trn_type="TRN2")
    x = nc.dram_tensor("x", [n, shard], mybir.dt.float32, kind="ExternalInput")
    y = nc.dram_tensor("y", [n, shard], mybir.dt.float32, kind="ExternalOutput")
    with nc.semaphore("dma_sem") as sem, nc.Block() as block:

        @block.sync
        def _(sync):
            for dst, src, cnt in blocks:
                sync.dma_start(
                    y[dst : dst + cnt, :], x[src : src + cnt, :]
                ).then_inc(sem, 16)
            sync.wait_ge(sem, 16 * len(blocks))

    return nc


def _make_runner(nc, n_cores, donate):
    """Jitted SPMD runner: x_global (n_cores*n, shard) -> y_global, sharded
    row-wise across cores.  Mirrors concourse.bass2jax.run_bass_via_pjrt but
    caches the jitted fn and (optionally) donates x so XLA aliases it to y."""
    import jax
    from jax.sharding import Mesh, NamedSharding, PartitionSpec

    from jax.experimental.shard_map import shard_map
    from concourse import mybir
    from concourse.bass2jax import (
        _bass_exec_p,
        install_neuronx_cc_hook,
        partition_id_tensor,
    )
    import concourse.mybir as _mybir

    install_neuronx_cc_hook()

    partition_name = nc.partition_id_tensor.name if nc.partition_id_tensor else None
    in_names, out_names, out_avals = [], [], []
    for alloc in nc.m.functions[0].allocations:
        if not isinstance(alloc, _mybir.MemoryLocationSet):
            continue
        name = alloc.memorylocations[0].name
        if alloc.kind == "ExternalInput":
            if name != partition_name:
                in_names.append(name)
        elif alloc.kind == "ExternalOutput":
            out_names.append(name)
            out_avals.append(
                jax.core.ShapedArray(tuple(alloc.tensor_shape), mybir.dt.np(alloc.dtype))
            )
    assert in_names == ["x"] and out_names == ["y"], (in_names, out_names)
    bind_in_names = tuple(in_names) + ((partition_name,) if partition_name else ())

    def _body(xarg):
        operands = [xarg]
        if partition_name is not None:
            operands.append(partition_id_tensor())
        outs = _bass_exec_p.bind(
            *operands,
            out_avals=tuple(out_avals),
            in_names=bind_in_names,
            out_names=tuple(out_names),
            lowering_input_output_aliases=(),
            sim_require_finite=True,
            sim_require_nnan=True,
            nc=nc,
        )
        return outs[0]

    devices = jax.devices()[:n_cores]
    assert len(devices) == n_cores, f"need {n_cores} devices, have {len(jax.devices())}"
    mesh = Mesh(np.asarray(devices), ("core",))
    spec = PartitionSpec("core")
    sharded = jax.jit(
        shard_map(_body, mesh=mesh, in_specs=(spec,), out_specs=spec, check_rep=False),
        donate_argnums=(0,) if donate else (),
        keep_unused=True,
    )
    sharding = NamedSharding(mesh, spec)

    def run(x_global: np.ndarray) -> np.ndarray:
        xdev = jax.device_put(x_global, sharding)
        out = jax.block_until_ready(sharded(xdev))
        return np.asarray(out)

    return run


def _get_runner(kind, blocks, n, shard):
    key = (kind, blocks, n, shard)
    if key not in _RUNNERS:
        if kind == "swap":
            nc = _build_swap_nc(blocks, n, shard)
            _RUNNERS[key] = _make_runner(nc, N_CORES, donate=True)
        else:
            nc = _build_copy_nc(blocks, n, shard)
            _RUNNERS[key] = _make_runner(nc, N_CORES, donate=False)
    return _RUNNERS[key]


def _shard_columns(x, n_cores):
    """(n, batch) -> (n_cores*n, batch//n_cores): core c gets columns
    [c*shard, (c+1)*shard), stacked along axis 0."""
    n, batch = x.shape
    shard = batch // n_cores
    return (
        np.ascontiguousarray(
            x.reshape(n, n_cores, shard).transpose(1, 0, 2)
        ).reshape(n_cores * n, shard),
        shard,
    )


def _unshard_columns(y_global, n, batch, n_cores):
    shard = batch // n_cores
    return np.ascontiguousarray(
        y_global.reshape(n_cores, n, shard).transpose(1, 0, 2)
    ).reshape(n, batch)


def kernel(U: np.ndarray, x: np.ndarray) -> np.ndarray:
    U = np.asarray(U)
    x = np.asarray(x)
    n, batch = x.shape

    # out[i] = x[perm[i]]  <=>  U[i, perm[i]] == 1 for a permutation matrix
    perm = np.argmax(U, axis=1)
    is_perm = (
        U.shape == (n, n)
        and float(U.sum(dtype=np.float64)) == float(n)
        and bool((U[np.arange(n), perm] == 1.0).all())
        and len(np.unique(perm)) == n
    )
    if not is_perm or batch % N_CORES != 0 or x.dtype != np.float32:
        # generic fallback (never taken for the CNOT problem)
        return np.asarray(U.astype(np.float64) @ x.astype(np.float64), dtype=x.dtype)

    blocks = _perm_to_blocks(perm)
    x_global, shard = _shard_columns(x, N_CORES)

    moving_bytes = sum(cnt for d, s, cnt in blocks if d != s) * shard * 4
    use_swap = 0 < moving_bytes <= SBUF_BUDGET_BYTES
    if moving_bytes == 0:
        return x.copy()

    if use_swap:
        try:
            run = _get_runner("swap", blocks, n, shard)
            y_global = run(x_global)
        except Exception:
            y_global = None
        if y_global is not None:
            out = _unshard_columns(y_global, n, batch, N_CORES)
            # verify the donated-buffer aliasing actually preserved identity rows
            ok = all(
                np.array_equal(out[d : d + cnt], x[d : d + cnt])
                for d, s, cnt in blocks
                if d == s
            )
            if ok:
                return out

    try:
        run = _get_runner("copy", blocks, n, shard)
        y_global = run(x_global)
    except Exception:
        # final fallback: the blessed SPMD path (also handles native NRT
        # environments where the jitted PJRT runner is unavailable)
        from concourse.bass_utils import run_bass_kernel_spmd

        nc = _build_copy_nc(blocks, n, shard)
        in_maps = [
            {"x": x_global[c * n : (c + 1) * n]} for c in range(N_CORES)
        ]
        res = run_bass_kernel_spmd(nc, in_maps, core_ids=list(range(N_CORES)))
        y_global = np.concatenate(
            [res.results[c]["y"] for c in range(N_CORES)], axis=0
        )
    return _unshard_columns(y_global, n, batch, N_CORES)

